# revision 5
# baseline (speedup 1.0000x reference)
"""Trainium2 Bass kernel: gated MoE residual block (two 3x3 convs, C=32).

  g  = gate * (gate > 0)                          # [B, C]
  h  = relu((conv3x3(x, w1) + b1) * g)
  h2 = relu((conv3x3(h, w2) + b2) * g)
  out = h2 + x

Sharding: data-parallel over batch. 16 images -> 8 cores x 2 images.

Device algorithm (fp8 DoubleRow edition):
  - x arrives host-packed in mod-4 row-interleaved fp8 layout x_il
    [128, 66, 258]: partition 32s+ci, slot t = row window t-1 (slots 0 and
    65 zero), col u = x col u-1 (zero halo cols 0, 257).
  - algebra: g is folded into the conv2 weights per image
    (w2g[o,i] = w2[o,i] * g[i]), so h' = relu(conv1(x)+b1) carries no gate
    and both epilogues are a single relu(psum + bias[p]) op; the final
    per-channel g[co] scale moves to the host combine (out = g*h2' + x).
  - conv as fp8 DoubleRow matmuls (cost-model rate: 0.5 cycles/row, two
    128-deep K-tiles per instruction). Per 4-row window t: 3 matmuls, one
    per dx, each pairing the "main" k-tile (window slot t) with the "wrap"
    k-tile (slot t+1) via the natural slot-stride slice
    x_il[:, t:t+2, dx:dx+W]; weights wv[:, 2dx:2dx+2, :].
  - two windows share one PSUM bank ([128, 2, 256] f32); the second
    window's first matmul uses start=False, relying on the bank-level
    ghost-zero of untouched bytes in a started bank.
  - epilogues relu(psum + b) are load-balanced between ScalarE and VectorE,
    writing fp8 h_il (conv1, +1 row phase so conv2 reuses the structure)
    or fp8 out_stage (conv2, +2 row phase, de-interleaved on host).
"""

import numpy as np
import ml_dtypes

import concourse.bass as bass
import concourse.tile as tile
from concourse import bacc, mybir

B, C, H, W = 16, 32, 256, 256
KW = 3
S = 4                 # row interleave factor (rows per window)
A = H // S            # 64 windows
WP = W + 2            # padded row width (zero cols 0 and 257)
NS = A + 2            # x_il/h_il slots: slot t = window t-1; 0 and 65 zero
NOS = A + 1           # out_stage slots (phase-2): slots 0..64
IMGS_PER_CORE = 2
N_CORES = 8
NV = 2 * KW           # weight k-tile stack: (main, wrap) x 3 dx
F32 = mybir.dt.float32
F8 = mybir.dt.float8e4
NPF8 = ml_dtypes.float8_e4m3
DR = mybir.MatmulPerfMode.DoubleRow
RELU = mybir.ActivationFunctionType.Relu


def _pack_weights(w: np.ndarray) -> np.ndarray:
    """w: [C_out, C_in, 3, 3] (OIHW) -> [NV, 128, 128] lhsT stack.

    Block (s, q) of main[dx] = w[:, :, s-q, dx].T   (0 <= s-q <= 2)
    Block (s, q) of wrap[dx] = w[:, :, 4+s-q, dx].T (0 <= 4+s-q <= 2)
    lhsT[(32s+ci), (32q+co)]; out row (window k at slot k+1) = 4k+1+q.
    """
    wv = np.zeros((NV, S * C, S * C), dtype=np.float32)
    for dx in range(KW):
        for q in range(S):
            for s in range(S):
                if 0 <= s - q <= 2:
                    wv[2 * dx, 32 * s:32 * s + 32, 32 * q:32 * q + 32] = \
                        w[:, :, s - q, dx].T
                if 0 <= 4 + s - q <= 2:
                    wv[2 * dx + 1, 32 * s:32 * s + 32, 32 * q:32 * q + 32] = \
                        w[:, :, 4 + s - q, dx].T
    return wv


def _interleave_x(x: np.ndarray) -> np.ndarray:
    """x: [n, C, H, W] f32 -> x_il [n, 128, NS, WP] fp8 (zero halo baked)."""
    n = x.shape[0]
    x8 = x.astype(NPF8)
    ext = np.zeros((n, C, S * NS, W), dtype=NPF8)
    ext[:, :, S:S + H, :] = x8
    il = ext.reshape(n, C, NS, S, W).transpose(0, 3, 1, 2, 4) \
            .reshape(n, S * C, NS, W)
    x_il = np.zeros((n, S * C, NS, WP), dtype=NPF8)
    x_il[:, :, :, 1:1 + W] = il
    return np.ascontiguousarray(x_il)


def _deinterleave_out(dev: np.ndarray) -> np.ndarray:
    """dev: [n, 128, NOS, W] (out row 4(t-1)+2+q at partition 32q+co)
    -> [n, C, H, W] f32."""
    dev = np.asarray(dev).astype(np.float32)
    n = dev.shape[0]
    v = dev.reshape(n, S, C, NOS, W).transpose(0, 2, 3, 1, 4) \
           .reshape(n, C, S * NOS, W)
    return np.ascontiguousarray(v[:, :, 2:2 + H, :])


def _build_core_graph():
    nc = bacc.Bacc(None, target_bir_lowering=False, debug=False)

    xil_ext = nc.declare_dram_parameter(
        "xil", [IMGS_PER_CORE, S * C, NS, WP], F8, isOutput=False)
    wv1_ext = nc.declare_dram_parameter("wv1", [S * C, NV, S * C], F8, isOutput=False)
    wv2_ext = nc.declare_dram_parameter(
        "wv2", [IMGS_PER_CORE, S * C, NV, S * C], F8, isOutput=False)
    b1_ext = nc.declare_dram_parameter("b1t", [S * C, 1], F32, isOutput=False)
    b2_ext = nc.declare_dram_parameter("b2t", [S * C, 1], F32, isOutput=False)
    out_ext = nc.declare_dram_parameter(
        "out", [IMGS_PER_CORE, S * C, NOS, W], F8, isOutput=True)

    with tile.TileContext(nc) as tc:
        with (
            tc.tile_pool(name="const", bufs=1) as cpool,
            tc.tile_pool(name="xb", bufs=1) as xpool,
            tc.tile_pool(name="hb", bufs=1) as hpool,
            tc.tile_pool(name="os", bufs=1) as ospool,
            tc.tile_pool(name="ps", bufs=4, space=bass.MemorySpace.PSUM) as pspool,
        ):
            wv1_t = cpool.tile([S * C, NV, S * C], F8)
            wv2_ts = [cpool.tile([S * C, NV, S * C], F8, tag=f"wv2_{i}",
                                 name=f"wv2_{i}")
                      for i in range(IMGS_PER_CORE)]
            b1_t = cpool.tile([S * C, 1], F32)
            b2_t = cpool.tile([S * C, 1], F32)

            # PE warm-up: dummy matmuls start the p-state ramp while the
            # first x chunk and weights stream in (results never read)
            warm = cpool.tile([S * C, 512], mybir.dt.bfloat16, tag="warm")
            nc.vector.memset(warm[:], 0.0)
            wps = pspool.tile([S * C, 4, W], F32, tag="ps")
            for _ in range(5):
                nc.tensor.matmul(
                    wps[:, 0, :], warm[:, 0:S * C], warm[:, 0:256],
                    start=True, stop=True, skip_group_check=True)

            x_ts = [xpool.tile([S * C, NS, WP], F8, tag=f"x_{i}", name=f"x_{i}")
                    for i in range(IMGS_PER_CORE)]
            h_ts = [hpool.tile([S * C, NS, WP], F8, tag=f"h_{i}", name=f"h_{i}")
                    for i in range(IMGS_PER_CORE)]
            o_ts = [ospool.tile([S * C, NOS, W], F8, tag=f"o_{i}", name=f"o_{i}")
                    for i in range(IMGS_PER_CORE)]

            # ---- input DMAs, issued in first-need order ----
            # SP: x image 0 (small head chunk so PE can start early), then
            # the rest; ACT issues wv1 (needed as early as x slots 0:4)
            nc.scalar.dma_start(out=wv1_t[:], in_=wv1_ext[:])
            nc.sync.dma_start(out=x_ts[0][:, 0:5, :], in_=xil_ext[0, :, 0:5, :])
            nc.gpsimd.dma_start(out=wv2_ts[0][:], in_=wv2_ext[0])
            nc.gpsimd.dma_start(out=b1_t[:], in_=b1_ext[:])
            nc.gpsimd.dma_start(out=b2_t[:], in_=b2_ext[:])
            nc.gpsimd.dma_start(out=wv2_ts[1][:], in_=wv2_ext[1])
            for c0, c1 in ((5, 14), (14, 27), (27, 45), (45, NS)):
                nc.sync.dma_start(out=x_ts[0][:, c0:c1, :],
                                  in_=xil_ext[0, :, c0:c1, :])
            for c0, c1 in ((0, 14), (14, 27), (27, 45), (45, NS)):
                nc.sync.dma_start(out=x_ts[1][:, c0:c1, :],
                                  in_=xil_ext[1, :, c0:c1, :])

            # ---- h halo zeroing (once per buffer; epilogues never dirty it)
            for h_t in h_ts:
                nc.gpsimd.memset(h_t[:, 0, :], 0.0)                  # rows <0
                nc.gpsimd.memset(h_t[3 * C:4 * C, A, :], 0.0)        # row 256
                nc.gpsimd.memset(h_t[:, A + 1, :], 0.0)              # rows >256
                nc.gpsimd.memset(h_t[:, :, 0], 0.0)                  # col halo
                nc.gpsimd.memset(h_t[:, :, WP - 1], 0.0)

            # greedy ACT/DVE load balancer for epilogue ops
            eng_load = {"act": 1783.0, "dve": 0.0}

            def epilogue(dst_ap, src_ap, bias_t, nel):
                cost_a = nel * 0.833 + 185.0
                cost_d = nel * 1.042 + 125.0
                if eng_load["act"] + cost_a <= eng_load["dve"] + cost_d:
                    eng_load["act"] += cost_a
                    nc.scalar.activation(dst_ap, src_ap, RELU,
                                         bias=bias_t, scale=1.0)
                else:
                    eng_load["dve"] += cost_d
                    nc.vector.tensor_scalar(
                        dst_ap, src_ap, bias_t, 0.0,
                        mybir.AluOpType.add, mybir.AluOpType.max)

            def conv(src_t, wv_t, bias_t, dst_t, is_conv1, img):
                """65 windows t=0..64 (k0 = t-1); 4 windows (2 PSUM banks)
                per tile, one epilogue op per tile."""
                t = 0
                while t <= A:
                    jn = min(4, A + 1 - t)
                    ps = pspool.tile([S * C, 4, W], F32, tag="ps", name="ps")
                    for j in range(jn):
                        for dx in range(KW):
                            nc.tensor.matmul(
                                ps[:, j, :],
                                wv_t[:, 2 * dx:2 * dx + 2, :],
                                src_t[:, t + j:t + j + 2, dx:dx + W],
                                # start marks each 2KB bank's zero region;
                                # the odd window of a bank ghost-zeroes
                                start=(j % 2 == 0 and dx == 0),
                                stop=(dx == KW - 1 and
                                      (j % 2 == 1 or j == jn - 1)),
                                perf_mode=DR, skip_group_check=True)
                    if is_conv1:
                        # h_il[:, t, 1:257] <- relu(ps[:, j] + b1)
                        if t == 0:
                            # slot 0: only q=3 (row 0) is real; keep the
                            # zero halo at partitions 0:96
                            epilogue(dst_t[3 * C:4 * C, 0, 1:1 + W],
                                     ps[3 * C:4 * C, 0, :], bias_t[3 * C:4 * C, 0:1], W)
                            epilogue(dst_t[:, 1:4, 1:1 + W],
                                     ps[:, 1:4, :], bias_t[:, 0:1], 3 * W)
                        elif t == A:
                            # slot 64: only q<3 (rows 253..255) are real
                            epilogue(dst_t[0:3 * C, A, 1:1 + W],
                                     ps[0:3 * C, 0, :], bias_t[0:3 * C, 0:1], W)
                        else:
                            epilogue(dst_t[:, t:t + jn, 1:1 + W],
                                     ps[:, 0:jn, :], bias_t[:, 0:1], jn * W)
                    else:
                        # out_stage[:, t, :] <- relu(ps + b2); edge rows are
                        # garbage the host never reads
                        epilogue(dst_t[:, t:t + jn, :], ps[:, 0:jn, :],
                                 bias_t[:, 0:1], jn * W)
                        # store completed slots (gpsimd SWDGE path keeps the
                        # global HWDGE free for x loads)
                        hi = t + jn
                        for s0, s1 in ((0, 16), (16, 32), (32, 48), (48, 64),
                                       (64, NOS)):
                            if hi == s1 or (hi == NOS and s0 < NOS <= s1):
                                nc.gpsimd.dma_start(
                                    out=out_ext[img, :, s0:min(s1, NOS), :],
                                    in_=dst_t[:, s0:min(s1, NOS), :])
                    t += jn

            for img in range(IMGS_PER_CORE):
                conv(x_ts[img], wv1_t, b1_t, h_ts[img], True, img)
                conv(h_ts[img], wv2_ts[img], b2_t, o_ts[img], False, img)

    nc.compile()
    return nc


def _host_prep(x, gate_values, w1, b1, w2, b2):
    x = np.ascontiguousarray(np.asarray(x, dtype=np.float32))
    gate_values = np.asarray(gate_values, dtype=np.float32)
    w1 = np.asarray(w1, dtype=np.float32)
    b1 = np.asarray(b1, dtype=np.float32)
    w2 = np.asarray(w2, dtype=np.float32)
    b2 = np.asarray(b2, dtype=np.float32)

    g = gate_values * (gate_values > 0)                      # [B, C]
    wv1 = np.ascontiguousarray(
        _pack_weights(w1).transpose(1, 0, 2)).astype(NPF8)
    b1t = np.ascontiguousarray(np.tile(b1, S)[:, None]).astype(np.float32)
    b2t = np.ascontiguousarray(np.tile(b2, S)[:, None]).astype(np.float32)

    in_maps = []
    for core in range(N_CORES):
        sl = slice(core * IMGS_PER_CORE, (core + 1) * IMGS_PER_CORE)
        wv2 = np.stack([
            np.ascontiguousarray(
                _pack_weights(w2 * g[core * IMGS_PER_CORE + i][None, :, None, None])
                .transpose(1, 0, 2)).astype(NPF8)
            for i in range(IMGS_PER_CORE)])
        in_maps.append({
            "xil": _interleave_x(x[sl]),
            "wv1": wv1, "wv2": wv2,
            "b1t": b1t, "b2t": b2t,
        })
    return in_maps


_NC_CACHE = None


def _get_graph():
    global _NC_CACHE
    if _NC_CACHE is None:
        _NC_CACHE = _build_core_graph()
    return _NC_CACHE


def kernel(x, gate_values, w1, b1, w2, b2, _trace=False, **_ignored):
    from concourse.bass_utils import run_bass_kernel_spmd

    nc = _get_graph()
    in_maps = _host_prep(x, gate_values, w1, b1, w2, b2)
    res = run_bass_kernel_spmd(
        nc, in_maps, core_ids=list(range(N_CORES)), trace=_trace)
    outs = [_deinterleave_out(res.results[i]["out"]) for i in range(N_CORES)]
    full = np.concatenate(outs, axis=0)                      # h2' f32
    gate_values = np.asarray(gate_values, dtype=np.float32)
    g = gate_values * (gate_values > 0)
    full *= g[:, :, None, None]
    full += np.asarray(x, dtype=np.float32)
    if _trace:
        return full, res
    return full


# revision 6
# speedup vs baseline: 1.0431x; 1.0431x over previous
"""Trainium2 Bass kernel: gated MoE residual block (two 3x3 convs, C=32).

  g  = gate * (gate > 0)                          # [B, C]
  h  = relu((conv3x3(x, w1) + b1) * g)
  h2 = relu((conv3x3(h, w2) + b2) * g)
  out = h2 + x

Sharding: data-parallel over batch. 16 images -> 8 cores x 2 images.

Device algorithm (fp8 DoubleRow edition):
  - x arrives host-packed in mod-4 row-interleaved fp8 layout x_il
    [128, 66, 258]: partition 32s+ci, slot t = row window t-1 (slots 0 and
    65 zero), col u = x col u-1 (zero halo cols 0, 257).
  - algebra: g is folded into the conv2 weights per image
    (w2g[o,i] = w2[o,i] * g[i]), so h' = relu(conv1(x)+b1) carries no gate
    and both epilogues are a single relu(psum + bias[p]) op; the final
    per-channel g[co] scale moves to the host combine (out = g*h2' + x).
  - conv as fp8 DoubleRow matmuls (cost-model rate: 0.5 cycles/row, two
    128-deep K-tiles per instruction). Per 4-row window t: 3 matmuls, one
    per dx, each pairing the "main" k-tile (window slot t) with the "wrap"
    k-tile (slot t+1) via the natural slot-stride slice
    x_il[:, t:t+2, dx:dx+W]; weights wv[:, 2dx:2dx+2, :].
  - two windows share one PSUM bank ([128, 2, 256] f32); the second
    window's first matmul uses start=False, relying on the bank-level
    ghost-zero of untouched bytes in a started bank.
  - epilogues relu(psum + b) are load-balanced between ScalarE and VectorE,
    writing fp8 h_il (conv1, +1 row phase so conv2 reuses the structure)
    or fp8 out_stage (conv2, +2 row phase, de-interleaved on host).
"""

import numpy as np
import ml_dtypes

import concourse.bass as bass
import concourse.tile as tile
from concourse import bacc, mybir

B, C, H, W = 16, 32, 256, 256
KW = 3
S = 4                 # row interleave factor (rows per window)
A = H // S            # 64 windows
WP = W + 2            # padded row width (zero cols 0 and 257)
NS = A + 2            # x_il/h_il slots: slot t = window t-1; 0 and 65 zero
NOS = A + 1           # out_stage slots (phase-2): slots 0..64
IMGS_PER_CORE = 2
N_CORES = 8
NV = 2 * KW           # weight k-tile stack: (main, wrap) x 3 dx
F32 = mybir.dt.float32
F8 = mybir.dt.float8e4
NPF8 = ml_dtypes.float8_e4m3
DR = mybir.MatmulPerfMode.DoubleRow
RELU = mybir.ActivationFunctionType.Relu


def _pack_weights(w: np.ndarray) -> np.ndarray:
    """w: [C_out, C_in, 3, 3] (OIHW) -> [NV, 128, 128] lhsT stack.

    Block (s, q) of main[dx] = w[:, :, s-q, dx].T   (0 <= s-q <= 2)
    Block (s, q) of wrap[dx] = w[:, :, 4+s-q, dx].T (0 <= 4+s-q <= 2)
    lhsT[(32s+ci), (32q+co)]; out row (window k at slot k+1) = 4k+1+q.
    """
    wv = np.zeros((NV, S * C, S * C), dtype=np.float32)
    for dx in range(KW):
        for q in range(S):
            for s in range(S):
                if 0 <= s - q <= 2:
                    wv[2 * dx, 32 * s:32 * s + 32, 32 * q:32 * q + 32] = \
                        w[:, :, s - q, dx].T
                if 0 <= 4 + s - q <= 2:
                    wv[2 * dx + 1, 32 * s:32 * s + 32, 32 * q:32 * q + 32] = \
                        w[:, :, 4 + s - q, dx].T
    return wv


def _interleave_x(x: np.ndarray) -> np.ndarray:
    """x: [n, C, H, W] f32 -> x_il [n, 128, NS, WP] fp8 (zero halo baked)."""
    n = x.shape[0]
    x8 = x.astype(NPF8)
    ext = np.zeros((n, C, S * NS, W), dtype=NPF8)
    ext[:, :, S:S + H, :] = x8
    il = ext.reshape(n, C, NS, S, W).transpose(0, 3, 1, 2, 4) \
            .reshape(n, S * C, NS, W)
    x_il = np.zeros((n, S * C, NS, WP), dtype=NPF8)
    x_il[:, :, :, 1:1 + W] = il
    return np.ascontiguousarray(x_il)


def _deinterleave_out(dev: np.ndarray) -> np.ndarray:
    """dev: [n, 128, NOS, W] (out row 4(t-1)+2+q at partition 32q+co)
    -> [n, C, H, W] f32."""
    dev = np.asarray(dev).astype(np.float32)
    n = dev.shape[0]
    v = dev.reshape(n, S, C, NOS, W).transpose(0, 2, 3, 1, 4) \
           .reshape(n, C, S * NOS, W)
    return np.ascontiguousarray(v[:, :, 2:2 + H, :])


def _build_core_graph():
    nc = bacc.Bacc(None, target_bir_lowering=False, debug=False)

    xil_ext = nc.declare_dram_parameter(
        "xil", [IMGS_PER_CORE, S * C, NS, WP], F8, isOutput=False)
    wv1_ext = nc.declare_dram_parameter("wv1", [S * C, NV, S * C], F8, isOutput=False)
    wv2_ext = nc.declare_dram_parameter(
        "wv2", [IMGS_PER_CORE, S * C, NV, S * C], F8, isOutput=False)
    b1_ext = nc.declare_dram_parameter("b1t", [S * C, 1], F32, isOutput=False)
    b2_ext = nc.declare_dram_parameter("b2t", [S * C, 1], F32, isOutput=False)
    out_ext = nc.declare_dram_parameter(
        "out", [IMGS_PER_CORE, S * C, NOS, W], F8, isOutput=True)

    with tile.TileContext(nc) as tc:
        with (
            tc.tile_pool(name="const", bufs=1) as cpool,
            tc.tile_pool(name="xb", bufs=1) as xpool,
            tc.tile_pool(name="hb", bufs=1) as hpool,
            tc.tile_pool(name="os", bufs=1) as ospool,
            tc.tile_pool(name="ps", bufs=4, space=bass.MemorySpace.PSUM) as pspool,
        ):
            wv1_t = cpool.tile([S * C, NV, S * C], F8)
            wv2_ts = [cpool.tile([S * C, NV, S * C], F8, tag=f"wv2_{i}",
                                 name=f"wv2_{i}")
                      for i in range(IMGS_PER_CORE)]
            b1_t = cpool.tile([S * C, 1], F32)
            b2_t = cpool.tile([S * C, 1], F32)

            # PE warm-up: dummy matmuls start the p-state ramp while the
            # first x chunk and weights stream in (results never read)
            warm = cpool.tile([S * C, 512], mybir.dt.bfloat16, tag="warm")
            nc.vector.memset(warm[:], 0.0)
            wps = pspool.tile([S * C, 4, W], F32, tag="ps")
            for _ in range(7):
                nc.tensor.matmul(
                    wps[:, 0, :], warm[:, 0:S * C], warm[:, 0:256],
                    start=True, stop=True, skip_group_check=True)

            x_ts = [xpool.tile([S * C, NS, WP], F8, tag=f"x_{i}", name=f"x_{i}")
                    for i in range(IMGS_PER_CORE)]
            h_ts = [hpool.tile([S * C, NS, WP], F8, tag=f"h_{i}", name=f"h_{i}")
                    for i in range(IMGS_PER_CORE)]
            o_ts = [ospool.tile([S * C, NOS, W], F8, tag=f"o_{i}", name=f"o_{i}")
                    for i in range(IMGS_PER_CORE)]

            # ---- input DMAs, issued in first-need order ----
            # Pool issues wv1 first (SWDGE path, lands ~1.5us); SP streams a
            # small x head chunk so the first window can start early
            nc.gpsimd.dma_start(out=wv1_t[:], in_=wv1_ext[:])
            nc.sync.dma_start(out=x_ts[0][:, 0:3, :], in_=xil_ext[0, :, 0:3, :])
            nc.gpsimd.dma_start(out=b1_t[:], in_=b1_ext[:])
            nc.gpsimd.dma_start(out=wv2_ts[0][:], in_=wv2_ext[0])
            nc.gpsimd.dma_start(out=b2_t[:], in_=b2_ext[:])
            nc.gpsimd.dma_start(out=wv2_ts[1][:], in_=wv2_ext[1])
            for c0, c1 in ((3, 14), (14, 27), (27, 45), (45, NS)):
                nc.sync.dma_start(out=x_ts[0][:, c0:c1, :],
                                  in_=xil_ext[0, :, c0:c1, :])
            for c0, c1 in ((0, 14), (14, 27), (27, 45), (45, NS)):
                nc.sync.dma_start(out=x_ts[1][:, c0:c1, :],
                                  in_=xil_ext[1, :, c0:c1, :])

            # ---- h halo zeroing (once per buffer; epilogues never dirty it)
            for h_t in h_ts:
                nc.gpsimd.memset(h_t[:, 0, :], 0.0)                  # rows <0
                nc.gpsimd.memset(h_t[3 * C:4 * C, A, :], 0.0)        # row 256
                nc.gpsimd.memset(h_t[:, A + 1, :], 0.0)              # rows >256
                nc.gpsimd.memset(h_t[:, :, 0], 0.0)                  # col halo
                nc.gpsimd.memset(h_t[:, :, WP - 1], 0.0)

            # greedy ACT/DVE load balancer for epilogue ops
            eng_load = {"act": 1783.0, "dve": 0.0}

            def epilogue(dst_ap, src_ap, bias_t, nel):
                cost_a = nel * 0.833 + 185.0
                cost_d = nel * 1.042 + 125.0
                if eng_load["act"] + cost_a <= eng_load["dve"] + cost_d:
                    eng_load["act"] += cost_a
                    nc.scalar.activation(dst_ap, src_ap, RELU,
                                         bias=bias_t, scale=1.0)
                else:
                    eng_load["dve"] += cost_d
                    nc.vector.tensor_scalar(
                        dst_ap, src_ap, bias_t, 0.0,
                        mybir.AluOpType.add, mybir.AluOpType.max)

            def conv(src_t, wv_t, bias_t, dst_t, is_conv1, img, tail=False):
                """65 windows t=0..64 (k0 = t-1); 4 windows (2 PSUM banks)
                per tile, one epilogue op per tile."""
                t = 0
                while t <= A:
                    jn = min(4, A + 1 - t)
                    ps = pspool.tile([S * C, 4, W], F32, tag="ps", name="ps")
                    for j in range(jn):
                        for dx in range(KW):
                            nc.tensor.matmul(
                                ps[:, j, :],
                                wv_t[:, 2 * dx:2 * dx + 2, :],
                                src_t[:, t + j:t + j + 2, dx:dx + W],
                                # start marks each 2KB bank's zero region;
                                # the odd window of a bank ghost-zeroes
                                start=(j % 2 == 0 and dx == 0),
                                stop=(dx == KW - 1 and
                                      (j % 2 == 1 or j == jn - 1)),
                                perf_mode=DR, skip_group_check=True)
                    if is_conv1:
                        # h_il[:, t, 1:257] <- relu(ps[:, j] + b1)
                        if t == 0:
                            # slot 0: only q=3 (row 0) is real; keep the
                            # zero halo at partitions 0:96
                            epilogue(dst_t[3 * C:4 * C, 0, 1:1 + W],
                                     ps[3 * C:4 * C, 0, :], bias_t[3 * C:4 * C, 0:1], W)
                            epilogue(dst_t[:, 1:4, 1:1 + W],
                                     ps[:, 1:4, :], bias_t[:, 0:1], 3 * W)
                        elif t == A:
                            # slot 64: only q<3 (rows 253..255) are real
                            epilogue(dst_t[0:3 * C, A, 1:1 + W],
                                     ps[0:3 * C, 0, :], bias_t[0:3 * C, 0:1], W)
                        else:
                            epilogue(dst_t[:, t:t + jn, 1:1 + W],
                                     ps[:, 0:jn, :], bias_t[:, 0:1], jn * W)
                    else:
                        # out_stage[:, t, :] <- relu(ps + b2); edge rows are
                        # garbage the host never reads
                        if tail and t >= 56 and jn == 4:
                            # split across both engines for a fast drain
                            epilogue(dst_t[:, t:t + 2, :], ps[:, 0:2, :],
                                     bias_t[:, 0:1], 2 * W)
                            epilogue(dst_t[:, t + 2:t + 4, :], ps[:, 2:4, :],
                                     bias_t[:, 0:1], 2 * W)
                        else:
                            epilogue(dst_t[:, t:t + jn, :], ps[:, 0:jn, :],
                                     bias_t[:, 0:1], jn * W)
                        # store completed slots (gpsimd SWDGE path keeps the
                        # global HWDGE free for x loads)
                        hi = t + jn
                        for s0, s1 in ((0, 16), (16, 32), (32, 48), (48, 56),
                                       (56, 60), (60, 64), (64, NOS)):
                            if hi == s1 or (hi == NOS and s0 < NOS <= s1):
                                eng = nc.sync if s0 >= 56 else nc.gpsimd
                                eng.dma_start(
                                    out=out_ext[img, :, s0:min(s1, NOS), :],
                                    in_=dst_t[:, s0:min(s1, NOS), :])
                    t += jn

            for img in range(IMGS_PER_CORE):
                conv(x_ts[img], wv1_t, b1_t, h_ts[img], True, img)
                conv(h_ts[img], wv2_ts[img], b2_t, o_ts[img], False, img,
                     tail=(img == IMGS_PER_CORE - 1))

    nc.compile()
    return nc


def _host_prep(x, gate_values, w1, b1, w2, b2):
    x = np.ascontiguousarray(np.asarray(x, dtype=np.float32))
    gate_values = np.asarray(gate_values, dtype=np.float32)
    w1 = np.asarray(w1, dtype=np.float32)
    b1 = np.asarray(b1, dtype=np.float32)
    w2 = np.asarray(w2, dtype=np.float32)
    b2 = np.asarray(b2, dtype=np.float32)

    g = gate_values * (gate_values > 0)                      # [B, C]
    wv1 = np.ascontiguousarray(
        _pack_weights(w1).transpose(1, 0, 2)).astype(NPF8)
    b1t = np.ascontiguousarray(np.tile(b1, S)[:, None]).astype(np.float32)
    b2t = np.ascontiguousarray(np.tile(b2, S)[:, None]).astype(np.float32)

    in_maps = []
    for core in range(N_CORES):
        sl = slice(core * IMGS_PER_CORE, (core + 1) * IMGS_PER_CORE)
        wv2 = np.stack([
            np.ascontiguousarray(
                _pack_weights(w2 * g[core * IMGS_PER_CORE + i][None, :, None, None])
                .transpose(1, 0, 2)).astype(NPF8)
            for i in range(IMGS_PER_CORE)])
        in_maps.append({
            "xil": _interleave_x(x[sl]),
            "wv1": wv1, "wv2": wv2,
            "b1t": b1t, "b2t": b2t,
        })
    return in_maps


_NC_CACHE = None


def _get_graph():
    global _NC_CACHE
    if _NC_CACHE is None:
        _NC_CACHE = _build_core_graph()
    return _NC_CACHE


def kernel(x, gate_values, w1, b1, w2, b2, _trace=False, **_ignored):
    from concourse.bass_utils import run_bass_kernel_spmd

    nc = _get_graph()
    in_maps = _host_prep(x, gate_values, w1, b1, w2, b2)
    res = run_bass_kernel_spmd(
        nc, in_maps, core_ids=list(range(N_CORES)), trace=_trace)
    outs = [_deinterleave_out(res.results[i]["out"]) for i in range(N_CORES)]
    full = np.concatenate(outs, axis=0)                      # h2' f32
    gate_values = np.asarray(gate_values, dtype=np.float32)
    g = gate_values * (gate_values > 0)
    full *= g[:, :, None, None]
    full += np.asarray(x, dtype=np.float32)
    if _trace:
        return full, res
    return full


# revision 8
# speedup vs baseline: 1.0444x; 1.0013x over previous
"""Trainium2 Bass kernel: gated MoE residual block (two 3x3 convs, C=32).

  g  = gate * (gate > 0)                          # [B, C]
  h  = relu((conv3x3(x, w1) + b1) * g)
  h2 = relu((conv3x3(h, w2) + b2) * g)
  out = h2 + x

Sharding: data-parallel over batch. 16 images -> 8 cores x 2 images.

Device algorithm (fp8 DoubleRow edition):
  - x arrives host-packed in mod-4 row-interleaved fp8 layout x_il
    [128, 66, 258]: partition 32s+ci, slot t = row window t-1 (slots 0 and
    65 zero), col u = x col u-1 (zero halo cols 0, 257).
  - algebra: g is folded into the conv2 weights per image
    (w2g[o,i] = w2[o,i] * g[i]), so h' = relu(conv1(x)+b1) carries no gate
    and both epilogues are a single relu(psum + bias[p]) op; the final
    per-channel g[co] scale moves to the host combine (out = g*h2' + x).
  - conv as fp8 DoubleRow matmuls (cost-model rate: 0.5 cycles/row, two
    128-deep K-tiles per instruction). Per 4-row window t: 3 matmuls, one
    per dx, each pairing the "main" k-tile (window slot t) with the "wrap"
    k-tile (slot t+1) via the natural slot-stride slice
    x_il[:, t:t+2, dx:dx+W]; weights wv[:, 2dx:2dx+2, :].
  - two windows share one PSUM bank ([128, 2, 256] f32); the second
    window's first matmul uses start=False, relying on the bank-level
    ghost-zero of untouched bytes in a started bank.
  - epilogues relu(psum + b) are load-balanced between ScalarE and VectorE,
    writing fp8 h_il (conv1, +1 row phase so conv2 reuses the structure)
    or fp8 out_stage (conv2, +2 row phase, de-interleaved on host).
"""

import numpy as np
import ml_dtypes

import concourse.bass as bass
import concourse.tile as tile
from concourse import bacc, mybir

B, C, H, W = 16, 32, 256, 256
KW = 3
S = 4                 # row interleave factor (rows per window)
A = H // S            # 64 windows
WP = W + 2            # padded row width (zero cols 0 and 257)
NS = A + 2            # x_il/h_il slots: slot t = window t-1; 0 and 65 zero
NOS = A + 1           # out_stage slots (phase-2): slots 0..64
IMGS_PER_CORE = 2
N_CORES = 8
NV = 2 * KW           # weight k-tile stack: (main, wrap) x 3 dx
F32 = mybir.dt.float32
F8 = mybir.dt.float8e4
NPF8 = ml_dtypes.float8_e4m3
DR = mybir.MatmulPerfMode.DoubleRow
RELU = mybir.ActivationFunctionType.Relu


def _pack_weights(w: np.ndarray) -> np.ndarray:
    """w: [C_out, C_in, 3, 3] (OIHW) -> [NV, 128, 128] lhsT stack.

    Block (s, q) of main[dx] = w[:, :, s-q, dx].T   (0 <= s-q <= 2)
    Block (s, q) of wrap[dx] = w[:, :, 4+s-q, dx].T (0 <= 4+s-q <= 2)
    lhsT[(32s+ci), (32q+co)]; out row (window k at slot k+1) = 4k+1+q.
    """
    wv = np.zeros((NV, S * C, S * C), dtype=np.float32)
    for dx in range(KW):
        for q in range(S):
            for s in range(S):
                if 0 <= s - q <= 2:
                    wv[2 * dx, 32 * s:32 * s + 32, 32 * q:32 * q + 32] = \
                        w[:, :, s - q, dx].T
                if 0 <= 4 + s - q <= 2:
                    wv[2 * dx + 1, 32 * s:32 * s + 32, 32 * q:32 * q + 32] = \
                        w[:, :, 4 + s - q, dx].T
    return wv


def _interleave_x(x: np.ndarray) -> np.ndarray:
    """x: [n, C, H, W] f32 -> x_il [n, 128, NS, WP] fp8 (zero halo baked)."""
    n = x.shape[0]
    x8 = x.astype(NPF8)
    ext = np.zeros((n, C, S * NS, W), dtype=NPF8)
    ext[:, :, S:S + H, :] = x8
    il = ext.reshape(n, C, NS, S, W).transpose(0, 3, 1, 2, 4) \
            .reshape(n, S * C, NS, W)
    x_il = np.zeros((n, S * C, NS, WP), dtype=NPF8)
    x_il[:, :, :, 1:1 + W] = il
    return np.ascontiguousarray(x_il)


def _deinterleave_out(dev: np.ndarray) -> np.ndarray:
    """dev: [n, 128, NOS, W] (out row 4(t-1)+2+q at partition 32q+co)
    -> [n, C, H, W] f32."""
    dev = np.asarray(dev).astype(np.float32)
    n = dev.shape[0]
    v = dev.reshape(n, S, C, NOS, W).transpose(0, 2, 3, 1, 4) \
           .reshape(n, C, S * NOS, W)
    return np.ascontiguousarray(v[:, :, 2:2 + H, :])


def _build_core_graph():
    nc = bacc.Bacc(None, target_bir_lowering=False, debug=False)

    xil_ext = nc.declare_dram_parameter(
        "xil", [IMGS_PER_CORE, S * C, NS, WP], F8, isOutput=False)
    wv1_ext = nc.declare_dram_parameter("wv1", [S * C, NV, S * C], F8, isOutput=False)
    wv2_ext = nc.declare_dram_parameter(
        "wv2", [IMGS_PER_CORE, S * C, NV, S * C], F8, isOutput=False)
    b1_ext = nc.declare_dram_parameter("b1t", [S * C, 1], F32, isOutput=False)
    b2_ext = nc.declare_dram_parameter("b2t", [S * C, 1], F32, isOutput=False)
    out_ext = nc.declare_dram_parameter(
        "out", [IMGS_PER_CORE, S * C, NOS, W], F8, isOutput=True)

    with tile.TileContext(nc) as tc:
        with (
            tc.tile_pool(name="const", bufs=1) as cpool,
            tc.tile_pool(name="xb", bufs=1) as xpool,
            tc.tile_pool(name="hb", bufs=1) as hpool,
            tc.tile_pool(name="os", bufs=1) as ospool,
            tc.tile_pool(name="ps", bufs=4, space=bass.MemorySpace.PSUM) as pspool,
        ):
            wv1_t = cpool.tile([S * C, NV, S * C], F8)
            wv2_ts = [cpool.tile([S * C, NV, S * C], F8, tag=f"wv2_{i}",
                                 name=f"wv2_{i}")
                      for i in range(IMGS_PER_CORE)]
            b1_t = cpool.tile([S * C, 1], F32)
            b2_t = cpool.tile([S * C, 1], F32)

            # PE warm-up: dummy matmuls start the p-state ramp while the
            # first x chunk and weights stream in (results never read)
            warm = cpool.tile([S * C, 256], mybir.dt.bfloat16, tag="warm")
            nc.vector.memset(warm[:], 0.0)
            wps = pspool.tile([S * C, 4, W], F32, tag="ps")
            for _ in range(7):
                nc.tensor.matmul(
                    wps[:, 0, :], warm[:, 0:S * C], warm[:],
                    start=True, stop=True, skip_group_check=True)

            x_ts = [xpool.tile([S * C, NS, WP], F8, tag=f"x_{i}", name=f"x_{i}")
                    for i in range(IMGS_PER_CORE)]
            h_ts = [hpool.tile([S * C, NS, WP], F8, tag=f"h_{i}", name=f"h_{i}")
                    for i in range(IMGS_PER_CORE)]
            o_ts = [ospool.tile([S * C, NOS, W], F8, tag=f"o_{i}", name=f"o_{i}")
                    for i in range(IMGS_PER_CORE)]

            # ---- input DMAs, issued in first-need order ----
            # Pool issues wv1 first (SWDGE path, lands ~1.5us); SP streams a
            # small x head chunk so the first window can start early
            nc.gpsimd.dma_start(out=wv1_t[:], in_=wv1_ext[:])
            nc.sync.dma_start(out=x_ts[0][:, 0:3, :], in_=xil_ext[0, :, 0:3, :])
            nc.gpsimd.dma_start(out=b1_t[:], in_=b1_ext[:])
            nc.gpsimd.dma_start(out=wv2_ts[0][:], in_=wv2_ext[0])
            nc.gpsimd.dma_start(out=b2_t[:], in_=b2_ext[:])
            nc.gpsimd.dma_start(out=wv2_ts[1][:], in_=wv2_ext[1])
            for c0, c1 in ((3, 14), (14, 27), (27, 45), (45, NS)):
                nc.sync.dma_start(out=x_ts[0][:, c0:c1, :],
                                  in_=xil_ext[0, :, c0:c1, :])
            for c0, c1 in ((0, 14), (14, 27), (27, 45), (45, NS)):
                nc.sync.dma_start(out=x_ts[1][:, c0:c1, :],
                                  in_=xil_ext[1, :, c0:c1, :])

            # ---- h halo zeroing (once per buffer; epilogues never dirty it)
            for h_t in h_ts:
                nc.gpsimd.memset(h_t[:, 0, :], 0.0)                  # rows <0
                nc.gpsimd.memset(h_t[3 * C:4 * C, A, :], 0.0)        # row 256
                nc.gpsimd.memset(h_t[:, A + 1, :], 0.0)              # rows >256
                nc.gpsimd.memset(h_t[:, :, 0], 0.0)                  # col halo
                nc.gpsimd.memset(h_t[:, :, WP - 1], 0.0)

            # greedy ACT/DVE load balancer for epilogue ops
            eng_load = {"act": 1783.0, "dve": 0.0}

            def epilogue(dst_ap, src_ap, bias_t, nel):
                cost_a = nel * 0.833 + 185.0
                cost_d = nel * 1.042 + 125.0
                if eng_load["act"] + cost_a <= eng_load["dve"] + cost_d:
                    eng_load["act"] += cost_a
                    nc.scalar.activation(dst_ap, src_ap, RELU,
                                         bias=bias_t, scale=1.0)
                else:
                    eng_load["dve"] += cost_d
                    nc.vector.tensor_scalar(
                        dst_ap, src_ap, bias_t, 0.0,
                        mybir.AluOpType.add, mybir.AluOpType.max)

            def conv(src_t, wv_t, bias_t, dst_t, is_conv1, img, tail=False):
                """65 windows t=0..64 (k0 = t-1); 4 windows (2 PSUM banks)
                per tile, one epilogue op per tile."""
                t = 0
                while t <= A:
                    jn = min(4, A + 1 - t)
                    ps = pspool.tile([S * C, 4, W], F32, tag="ps", name="ps")
                    for j in range(jn):
                        for dx in range(KW):
                            nc.tensor.matmul(
                                ps[:, j, :],
                                wv_t[:, 2 * dx:2 * dx + 2, :],
                                src_t[:, t + j:t + j + 2, dx:dx + W],
                                # start marks each 2KB bank's zero region;
                                # the odd window of a bank ghost-zeroes
                                start=(j % 2 == 0 and dx == 0),
                                stop=(dx == KW - 1 and
                                      (j % 2 == 1 or j == jn - 1)),
                                perf_mode=DR, skip_group_check=True)
                    if is_conv1:
                        # h_il[:, t, 1:257] <- relu(ps[:, j] + b1)
                        if t == 0:
                            # slot 0: only q=3 (row 0) is real; keep the
                            # zero halo at partitions 0:96
                            epilogue(dst_t[3 * C:4 * C, 0, 1:1 + W],
                                     ps[3 * C:4 * C, 0, :], bias_t[3 * C:4 * C, 0:1], W)
                            epilogue(dst_t[:, 1:4, 1:1 + W],
                                     ps[:, 1:4, :], bias_t[:, 0:1], 3 * W)
                        elif t == A:
                            # slot 64: only q<3 (rows 253..255) are real
                            epilogue(dst_t[0:3 * C, A, 1:1 + W],
                                     ps[0:3 * C, 0, :], bias_t[0:3 * C, 0:1], W)
                        else:
                            epilogue(dst_t[:, t:t + jn, 1:1 + W],
                                     ps[:, 0:jn, :], bias_t[:, 0:1], jn * W)
                    else:
                        # out_stage[:, t, :] <- relu(ps + b2); edge rows are
                        # garbage the host never reads
                        if tail and t >= 48 and jn == 4:
                            # split across both engines for a fast drain
                            epilogue(dst_t[:, t:t + 2, :], ps[:, 0:2, :],
                                     bias_t[:, 0:1], 2 * W)
                            epilogue(dst_t[:, t + 2:t + 4, :], ps[:, 2:4, :],
                                     bias_t[:, 0:1], 2 * W)
                        else:
                            epilogue(dst_t[:, t:t + jn, :], ps[:, 0:jn, :],
                                     bias_t[:, 0:1], jn * W)
                        # store completed slots (gpsimd SWDGE path keeps the
                        # global HWDGE free for x loads)
                        hi = t + jn
                        for s0, s1 in ((0, 16), (16, 32), (32, 48), (48, 56),
                                       (56, 60), (60, 64), (64, NOS)):
                            if hi == s1 or (hi == NOS and s0 < NOS <= s1):
                                eng = nc.sync if s0 >= 56 else nc.gpsimd
                                eng.dma_start(
                                    out=out_ext[img, :, s0:min(s1, NOS), :],
                                    in_=dst_t[:, s0:min(s1, NOS), :])
                    t += jn

            for img in range(IMGS_PER_CORE):
                conv(x_ts[img], wv1_t, b1_t, h_ts[img], True, img)
                conv(h_ts[img], wv2_ts[img], b2_t, o_ts[img], False, img,
                     tail=(img == IMGS_PER_CORE - 1))

    nc.compile()
    return nc


def _host_prep(x, gate_values, w1, b1, w2, b2):
    x = np.ascontiguousarray(np.asarray(x, dtype=np.float32))
    gate_values = np.asarray(gate_values, dtype=np.float32)
    w1 = np.asarray(w1, dtype=np.float32)
    b1 = np.asarray(b1, dtype=np.float32)
    w2 = np.asarray(w2, dtype=np.float32)
    b2 = np.asarray(b2, dtype=np.float32)

    g = gate_values * (gate_values > 0)                      # [B, C]
    wv1 = np.ascontiguousarray(
        _pack_weights(w1).transpose(1, 0, 2)).astype(NPF8)
    b1t = np.ascontiguousarray(np.tile(b1, S)[:, None]).astype(np.float32)
    b2t = np.ascontiguousarray(np.tile(b2, S)[:, None]).astype(np.float32)

    in_maps = []
    for core in range(N_CORES):
        sl = slice(core * IMGS_PER_CORE, (core + 1) * IMGS_PER_CORE)
        wv2 = np.stack([
            np.ascontiguousarray(
                _pack_weights(w2 * g[core * IMGS_PER_CORE + i][None, :, None, None])
                .transpose(1, 0, 2)).astype(NPF8)
            for i in range(IMGS_PER_CORE)])
        in_maps.append({
            "xil": _interleave_x(x[sl]),
            "wv1": wv1, "wv2": wv2,
            "b1t": b1t, "b2t": b2t,
        })
    return in_maps


_NC_CACHE = None


def _get_graph():
    global _NC_CACHE
    if _NC_CACHE is None:
        _NC_CACHE = _build_core_graph()
    return _NC_CACHE


def kernel(x, gate_values, w1, b1, w2, b2, _trace=False, **_ignored):
    from concourse.bass_utils import run_bass_kernel_spmd

    nc = _get_graph()
    in_maps = _host_prep(x, gate_values, w1, b1, w2, b2)
    res = run_bass_kernel_spmd(
        nc, in_maps, core_ids=list(range(N_CORES)), trace=_trace)
    outs = [_deinterleave_out(res.results[i]["out"]) for i in range(N_CORES)]
    full = np.concatenate(outs, axis=0)                      # h2' f32
    gate_values = np.asarray(gate_values, dtype=np.float32)
    g = gate_values * (gate_values > 0)
    full *= g[:, :, None, None]
    full += np.asarray(x, dtype=np.float32)
    if _trace:
        return full, res
    return full


# revision 9
# speedup vs baseline: 1.0518x; 1.0071x over previous
"""Trainium2 Bass kernel: gated MoE residual block (two 3x3 convs, C=32).

  g  = gate * (gate > 0)                          # [B, C]
  h  = relu((conv3x3(x, w1) + b1) * g)
  h2 = relu((conv3x3(h, w2) + b2) * g)
  out = h2 + x

Sharding: data-parallel over batch. 16 images -> 8 cores x 2 images.

Device algorithm (fp8 DoubleRow edition):
  - x arrives host-packed in mod-4 row-interleaved fp8 layout x_il
    [128, 66, 258]: partition 32s+ci, slot t = row window t-1 (slots 0 and
    65 zero), col u = x col u-1 (zero halo cols 0, 257).
  - algebra: g is folded into the conv2 weights per image
    (w2g[o,i] = w2[o,i] * g[i]), so h' = relu(conv1(x)+b1) carries no gate
    and both epilogues are a single relu(psum + bias[p]) op; the final
    per-channel g[co] scale moves to the host combine (out = g*h2' + x).
  - conv as fp8 DoubleRow matmuls (cost-model rate: 0.5 cycles/row, two
    128-deep K-tiles per instruction). Per 4-row window t: 3 matmuls, one
    per dx, each pairing the "main" k-tile (window slot t) with the "wrap"
    k-tile (slot t+1) via the natural slot-stride slice
    x_il[:, t:t+2, dx:dx+W]; weights wv[:, 2dx:2dx+2, :].
  - two windows share one PSUM bank ([128, 2, 256] f32); the second
    window's first matmul uses start=False, relying on the bank-level
    ghost-zero of untouched bytes in a started bank.
  - epilogues relu(psum + b) are load-balanced between ScalarE and VectorE,
    writing fp8 h_il (conv1, +1 row phase so conv2 reuses the structure)
    or fp8 out_stage (conv2, +2 row phase, de-interleaved on host).
"""

import numpy as np
import ml_dtypes

import concourse.bass as bass
import concourse.tile as tile
from concourse import bacc, mybir

B, C, H, W = 16, 32, 256, 256
KW = 3
S = 4                 # row interleave factor (rows per window)
A = H // S            # 64 windows
WP = W + 2            # padded row width (zero cols 0 and 257)
NS = A + 2            # x_il/h_il slots: slot t = window t-1; 0 and 65 zero
NOS = A + 1           # out_stage slots (phase-2): slots 0..64
IMGS_PER_CORE = 2
N_CORES = 8
NV = 2 * KW           # weight k-tile stack: (main, wrap) x 3 dx
F32 = mybir.dt.float32
F8 = mybir.dt.float8e4
NPF8 = ml_dtypes.float8_e4m3
DR = mybir.MatmulPerfMode.DoubleRow
RELU = mybir.ActivationFunctionType.Relu


def _pack_weights(w: np.ndarray) -> np.ndarray:
    """w: [C_out, C_in, 3, 3] (OIHW) -> [NV, 128, 128] lhsT stack.

    Block (s, q) of main[dx] = w[:, :, s-q, dx].T   (0 <= s-q <= 2)
    Block (s, q) of wrap[dx] = w[:, :, 4+s-q, dx].T (0 <= 4+s-q <= 2)
    lhsT[(32s+ci), (32q+co)]; out row (window k at slot k+1) = 4k+1+q.
    """
    wv = np.zeros((NV, S * C, S * C), dtype=np.float32)
    for dx in range(KW):
        for q in range(S):
            for s in range(S):
                if 0 <= s - q <= 2:
                    wv[2 * dx, 32 * s:32 * s + 32, 32 * q:32 * q + 32] = \
                        w[:, :, s - q, dx].T
                if 0 <= 4 + s - q <= 2:
                    wv[2 * dx + 1, 32 * s:32 * s + 32, 32 * q:32 * q + 32] = \
                        w[:, :, 4 + s - q, dx].T
    return wv


def _interleave_x(x: np.ndarray) -> np.ndarray:
    """x: [n, C, H, W] f32 -> x_il [n, 128, NS, WP] fp8 (zero halo baked)."""
    n = x.shape[0]
    x8 = x.astype(NPF8)
    ext = np.zeros((n, C, S * NS, W), dtype=NPF8)
    ext[:, :, S:S + H, :] = x8
    il = ext.reshape(n, C, NS, S, W).transpose(0, 3, 1, 2, 4) \
            .reshape(n, S * C, NS, W)
    x_il = np.zeros((n, S * C, NS, WP), dtype=NPF8)
    x_il[:, :, :, 1:1 + W] = il
    return np.ascontiguousarray(x_il)


def _deinterleave_out(dev: np.ndarray) -> np.ndarray:
    """dev: [n, 128, NOS, W] (out row 4(t-1)+2+q at partition 32q+co)
    -> [n, C, H, W] f32."""
    dev = np.asarray(dev).astype(np.float32)
    n = dev.shape[0]
    v = dev.reshape(n, S, C, NOS, W).transpose(0, 2, 3, 1, 4) \
           .reshape(n, C, S * NOS, W)
    return np.ascontiguousarray(v[:, :, 2:2 + H, :])


def _build_core_graph():
    nc = bacc.Bacc(None, target_bir_lowering=False, debug=False)

    xil_ext = nc.declare_dram_parameter(
        "xil", [IMGS_PER_CORE, S * C, NS, WP], F8, isOutput=False)
    wv1_ext = nc.declare_dram_parameter("wv1", [S * C, NV, S * C], F8, isOutput=False)
    wv2_ext = nc.declare_dram_parameter(
        "wv2", [IMGS_PER_CORE, S * C, NV, S * C], F8, isOutput=False)
    b1_ext = nc.declare_dram_parameter("b1t", [S * C, 1], F32, isOutput=False)
    b2_ext = nc.declare_dram_parameter("b2t", [S * C, 1], F32, isOutput=False)
    out_ext = nc.declare_dram_parameter(
        "out", [IMGS_PER_CORE, S * C, NOS, W], F8, isOutput=True)

    with tile.TileContext(nc) as tc:
        with (
            tc.tile_pool(name="const", bufs=1) as cpool,
            tc.tile_pool(name="xb", bufs=1) as xpool,
            tc.tile_pool(name="hb", bufs=1) as hpool,
            tc.tile_pool(name="os", bufs=1) as ospool,
            tc.tile_pool(name="ps", bufs=4, space=bass.MemorySpace.PSUM) as pspool,
        ):
            wv1_t = cpool.tile([S * C, NV, S * C], F8)
            wv2_ts = [cpool.tile([S * C, NV, S * C], F8, tag=f"wv2_{i}",
                                 name=f"wv2_{i}")
                      for i in range(IMGS_PER_CORE)]
            b1_t = cpool.tile([S * C, 1], F32)
            b2_t = cpool.tile([S * C, 1], F32)

            # PE warm-up: dummy matmuls start the p-state ramp while the
            # first x chunk and weights stream in (results never read)
            warm = cpool.tile([S * C, 256], mybir.dt.bfloat16, tag="warm")
            nc.vector.memset(warm[:], 0.0)
            wps = pspool.tile([S * C, 4, W], F32, tag="ps")
            for _ in range(7):
                nc.tensor.matmul(
                    wps[:, 0, :], warm[:, 0:S * C], warm[:],
                    start=True, stop=True, skip_group_check=True)

            x_ts = [xpool.tile([S * C, NS, WP], F8, tag=f"x_{i}", name=f"x_{i}")
                    for i in range(IMGS_PER_CORE)]
            h_ts = [hpool.tile([S * C, NS, WP], F8, tag=f"h_{i}", name=f"h_{i}")
                    for i in range(IMGS_PER_CORE)]
            o_ts = [ospool.tile([S * C, NOS, W], F8, tag=f"o_{i}", name=f"o_{i}")
                    for i in range(IMGS_PER_CORE)]

            # ---- input DMAs, issued in first-need order ----
            # Pool issues wv1 first (SWDGE path, lands ~1.5us); SP streams a
            # small x head chunk so the first window can start early
            nc.gpsimd.dma_start(out=wv1_t[:], in_=wv1_ext[:])
            nc.sync.dma_start(out=x_ts[0][:, 0:3, :], in_=xil_ext[0, :, 0:3, :])
            nc.gpsimd.dma_start(out=b1_t[:], in_=b1_ext[:])
            nc.gpsimd.dma_start(out=wv2_ts[0][:], in_=wv2_ext[0])
            nc.gpsimd.dma_start(out=b2_t[:], in_=b2_ext[:])
            nc.gpsimd.dma_start(out=wv2_ts[1][:], in_=wv2_ext[1])
            for c0, c1 in ((3, 14), (14, 27), (27, 45), (45, NS)):
                nc.sync.dma_start(out=x_ts[0][:, c0:c1, :],
                                  in_=xil_ext[0, :, c0:c1, :])
            for c0, c1 in ((0, 14), (14, 27), (27, 45), (45, NS)):
                nc.sync.dma_start(out=x_ts[1][:, c0:c1, :],
                                  in_=xil_ext[1, :, c0:c1, :])

            # ---- h halo zeroing (once per buffer; epilogues never dirty it)
            for h_t in h_ts:
                nc.gpsimd.memset(h_t[:, 0, :], 0.0)                  # rows <0
                nc.gpsimd.memset(h_t[3 * C:4 * C, A, :], 0.0)        # row 256
                nc.gpsimd.memset(h_t[:, A + 1, :], 0.0)              # rows >256
                nc.gpsimd.memset(h_t[:, :, 0], 0.0)                  # col halo
                nc.gpsimd.memset(h_t[:, :, WP - 1], 0.0)

            # greedy ACT/DVE load balancer for epilogue ops
            eng_load = {"act": 1783.0, "dve": 0.0}

            def epilogue(dst_ap, src_ap, bias_t, nel):
                cost_a = nel * 0.833 + 185.0
                cost_d = nel * 1.042 + 125.0
                if eng_load["act"] + cost_a <= eng_load["dve"] + cost_d:
                    eng_load["act"] += cost_a
                    nc.scalar.activation(dst_ap, src_ap, RELU,
                                         bias=bias_t, scale=1.0)
                else:
                    eng_load["dve"] += cost_d
                    nc.vector.tensor_scalar(
                        dst_ap, src_ap, bias_t, 0.0,
                        mybir.AluOpType.add, mybir.AluOpType.max)

            def conv(src_t, wv_t, bias_t, dst_t, is_conv1, img, tail=False):
                """65 windows t=0..64 (k0 = t-1); 4 windows (2 PSUM banks)
                per tile, one epilogue op per tile."""
                t = 0
                while t <= A:
                    jn = min(4, A + 1 - t)
                    ps = pspool.tile([S * C, 4, W], F32, tag="ps", name="ps")
                    for j in range(jn):
                        for dx in range(KW):
                            nc.tensor.matmul(
                                ps[:, j, :],
                                wv_t[:, 2 * dx:2 * dx + 2, :],
                                src_t[:, t + j:t + j + 2, dx:dx + W],
                                # start marks each 2KB bank's zero region;
                                # the odd window of a bank ghost-zeroes
                                start=(j % 2 == 0 and dx == 0),
                                stop=(dx == KW - 1 and
                                      (j % 2 == 1 or j == jn - 1)),
                                perf_mode=DR, skip_group_check=True)
                    if is_conv1:
                        # h_il[:, t, 1:257] <- relu(ps[:, j] + b1)
                        if t == 0:
                            # slot 0: only q=3 (row 0) is real; keep the
                            # zero halo at partitions 0:96
                            epilogue(dst_t[3 * C:4 * C, 0, 1:1 + W],
                                     ps[3 * C:4 * C, 0, :], bias_t[3 * C:4 * C, 0:1], W)
                            epilogue(dst_t[:, 1:4, 1:1 + W],
                                     ps[:, 1:4, :], bias_t[:, 0:1], 3 * W)
                        elif t == A:
                            # slot 64: only q<3 (rows 253..255) are real
                            epilogue(dst_t[0:3 * C, A, 1:1 + W],
                                     ps[0:3 * C, 0, :], bias_t[0:3 * C, 0:1], W)
                        else:
                            epilogue(dst_t[:, t:t + jn, 1:1 + W],
                                     ps[:, 0:jn, :], bias_t[:, 0:1], jn * W)
                    else:
                        # out_stage[:, t, :] <- relu(ps + b2); edge rows are
                        # garbage the host never reads
                        if tail and t >= 48 and jn == 4:
                            # split across both engines for a fast drain
                            epilogue(dst_t[:, t:t + 2, :], ps[:, 0:2, :],
                                     bias_t[:, 0:1], 2 * W)
                            epilogue(dst_t[:, t + 2:t + 4, :], ps[:, 2:4, :],
                                     bias_t[:, 0:1], 2 * W)
                        else:
                            epilogue(dst_t[:, t:t + jn, :], ps[:, 0:jn, :],
                                     bias_t[:, 0:1], jn * W)
                        # store completed slots (gpsimd SWDGE path keeps the
                        # global HWDGE free for x loads)
                        hi = t + jn
                        for s0, s1 in ((0, 16), (16, 32), (32, 48), (48, 56),
                                       (56, 60), (60, 64), (64, NOS)):
                            if hi == s1 or (hi == NOS and s0 < NOS <= s1):
                                # spread the tail stores across issue engines
                                # so their SEQ/HWDGE costs overlap
                                eng = (nc.scalar if s0 >= 64 else
                                       nc.sync if s0 >= 56 else nc.gpsimd)
                                eng.dma_start(
                                    out=out_ext[img, :, s0:min(s1, NOS), :],
                                    in_=dst_t[:, s0:min(s1, NOS), :])
                    t += jn

            for img in range(IMGS_PER_CORE):
                conv(x_ts[img], wv1_t, b1_t, h_ts[img], True, img)
                conv(h_ts[img], wv2_ts[img], b2_t, o_ts[img], False, img,
                     tail=(img == IMGS_PER_CORE - 1))

    nc.compile()
    return nc


def _host_prep(x, gate_values, w1, b1, w2, b2):
    x = np.ascontiguousarray(np.asarray(x, dtype=np.float32))
    gate_values = np.asarray(gate_values, dtype=np.float32)
    w1 = np.asarray(w1, dtype=np.float32)
    b1 = np.asarray(b1, dtype=np.float32)
    w2 = np.asarray(w2, dtype=np.float32)
    b2 = np.asarray(b2, dtype=np.float32)

    g = gate_values * (gate_values > 0)                      # [B, C]
    wv1 = np.ascontiguousarray(
        _pack_weights(w1).transpose(1, 0, 2)).astype(NPF8)
    b1t = np.ascontiguousarray(np.tile(b1, S)[:, None]).astype(np.float32)
    b2t = np.ascontiguousarray(np.tile(b2, S)[:, None]).astype(np.float32)

    in_maps = []
    for core in range(N_CORES):
        sl = slice(core * IMGS_PER_CORE, (core + 1) * IMGS_PER_CORE)
        wv2 = np.stack([
            np.ascontiguousarray(
                _pack_weights(w2 * g[core * IMGS_PER_CORE + i][None, :, None, None])
                .transpose(1, 0, 2)).astype(NPF8)
            for i in range(IMGS_PER_CORE)])
        in_maps.append({
            "xil": _interleave_x(x[sl]),
            "wv1": wv1, "wv2": wv2,
            "b1t": b1t, "b2t": b2t,
        })
    return in_maps


_NC_CACHE = None


def _get_graph():
    global _NC_CACHE
    if _NC_CACHE is None:
        _NC_CACHE = _build_core_graph()
    return _NC_CACHE


def kernel(x, gate_values, w1, b1, w2, b2, _trace=False, **_ignored):
    from concourse.bass_utils import run_bass_kernel_spmd

    nc = _get_graph()
    in_maps = _host_prep(x, gate_values, w1, b1, w2, b2)
    res = run_bass_kernel_spmd(
        nc, in_maps, core_ids=list(range(N_CORES)), trace=_trace)
    outs = [_deinterleave_out(res.results[i]["out"]) for i in range(N_CORES)]
    full = np.concatenate(outs, axis=0)                      # h2' f32
    gate_values = np.asarray(gate_values, dtype=np.float32)
    g = gate_values * (gate_values > 0)
    full *= g[:, :, None, None]
    full += np.asarray(x, dtype=np.float32)
    if _trace:
        return full, res
    return full
